# revision 43
# baseline (speedup 1.0000x reference)
"""Trainium2 Bass kernel for nn_BottleneckBlock (half-channel causal attention).

Contract: kernel(**inputs) takes the FULL unsharded inputs (as produced by the
problem's setup_inputs) and returns the FULL output, distributing work across
8 NeuronCores data-parallel over the (b, h, w) positions.

Per-core math (144 positions, seq N=64, C=256, 8 heads x 32):
  qkv = x @ qkv_w.T + qkv_b ; causal softmax(q k^T / sqrt(32) + rpb) @ v ; proj.

v2 design notes:
- Q/K/V projections run as fp8e4m3 DoubleRow matmuls (contraction 256 in one
  instruction); weights are pre-scaled by 64 to stay in fp8 normal range and
  the 64*64 product scale is folded into the softmax exp's scale argument.
- K's bias is dropped: its logit contribution is constant per query, which
  softmax cancels.  V's bias is folded into the projection bias on the host.
- PSUM (8 banks) budget: one rotating 2-bank buffer for q/k/v projections,
  2x 2-bank buffers for half-round S/AV tiles (S and AV share banks since S
  dies at exp), 2 banks for the output projection.
- S^T is computed per (position, head) in 64x64 blocks, 8-way packed on the
  PE array; exp runs as one scalar-engine instruction per half-round over a
  merged [128, 1024] PSUM view; the rel-pos-bias+causal mask multiply runs on
  the otherwise-idle GpSimd engine.
- AV uses an appended ones-column (value 64 to match V's scale) to produce
  softmax denominators for free; reciprocal+normalize are single merged DVE
  instructions.
- The [token, channel] -> [channel, token] transpose before the output
  projection runs on the DMA xbar (dma_start_transpose), not the PE.
- Output is written bf16 and upcast on the host.
"""

import os
import sys
from contextlib import ExitStack

import numpy as np

sys.path.insert(0, "/opt/trn_rl_repo")

import ml_dtypes

BF16 = ml_dtypes.bfloat16
FP8 = ml_dtypes.float8_e4m3

# Problem shape (hardcoded per spec)
B, T, CH, HS, WS = 2, 64, 512, 24, 24
HALF = CH // 2          # 256
HEADS = 8
HD = 32
SCALE = HD ** -0.5
NCORES = 8
NPOS = B * HS * WS      # 1152 positions
POS_PER_CORE = NPOS // NCORES   # 144
TOK = POS_PER_CORE * T  # 9216 tokens per core
NR = int(os.environ.get("KV2_NR", str(TOK // 512)))  # rounds (18 full)

# Feature bisect flags (env, read at import)
USE_DR = bool(int(os.environ.get("KV2_DR", "1")))      # fp8 DoubleRow qkv
USE_DMAT = bool(int(os.environ.get("KV2_DMAT", "1")))  # DMA-xbar transpose
USE_GPS = bool(int(os.environ.get("KV2_GPS", "1")))    # mask-mul on GpSimd
STAGE = int(os.environ.get("KV2_STAGE", "99"))         # crash-bisect stage
EXPSPLIT = bool(int(os.environ.get("KV2_EXPSPLIT", "0")))  # exp per psum bank
NOEXP = bool(int(os.environ.get("KV2_NOEXP", "0")))        # skip exp entirely
W8 = 64.0 if USE_DR else 1.0   # fp8 weight pre-scale

_BUILD_CACHE = {}


def _host_prep(x, rpb_table, qkv_w, qkv_b, proj_w, proj_b):
    """Build the 8 per-core input maps."""
    x = np.asarray(x, dtype=np.float32)
    qkv_w = np.asarray(qkv_w, dtype=np.float32)
    qkv_b = np.asarray(qkv_b, dtype=np.float32)
    proj_w = np.asarray(proj_w, dtype=np.float32)
    proj_b = np.asarray(proj_b, dtype=np.float32)
    rpb = np.asarray(rpb_table, dtype=np.float32)

    # ---- x transpose: (B,T,CH,H,W) attention half -> [c, B_*T] in fp8 ----
    b_part = x[:, :, HALF:]                       # (B,T,256,H,W)
    xt = np.transpose(b_part, (2, 0, 3, 4, 1))    # (256, B, H, W, T)
    xt = np.ascontiguousarray(xt).reshape(2, 128, NPOS * T)

    # ---- weights ----
    # wqk[p, kc, f]: f in [0,512) = (q heads then k heads); q rows pre-scaled
    # by softmax scale; everything scaled by W8 for fp8 range.
    wqk = np.empty((128, 2, 512), dtype=np.float32)
    for kc in range(2):
        wqk[:, kc, 0:256] = (qkv_w[0:256] * (SCALE * W8)).T[128 * kc: 128 * (kc + 1)]
        wqk[:, kc, 256:512] = (qkv_w[256:512] * W8).T[128 * kc: 128 * (kc + 1)]
    bq64 = np.stack(
        [qkv_b[0:128], qkv_b[128:256]], axis=1
    ).astype(np.float32) * (SCALE * W8)           # [128, 2]

    wv = np.empty((128, 2, 256), dtype=np.float32)
    for kc in range(2):
        wv[:, kc, :] = (qkv_w[512:768] * W8).T[128 * kc: 128 * (kc + 1)]

    # proj stays bf16; lhsT layout [p, w, oc]
    wp = np.empty((128, 2, 256), dtype=np.float32)
    for w in range(2):
        wp[:, w, :] = proj_w.T[128 * w: 128 * (w + 1)]
    bv = qkv_b[512:768]
    bp_full = proj_b + proj_w @ bv
    bp = np.stack([bp_full[0:128], bp_full[128:256]], axis=1).astype(np.float32)

    # ---- exp(bias) * causal mask, [key, query] layout ----
    pos = np.arange(T)
    rel = pos[None, :] - pos[:, None] + (T - 1)   # [i(q), j(k)] -> index
    bias = rpb[rel]                               # [q, k, heads]
    eb = np.exp(bias.transpose(2, 0, 1))          # [h, q, k]
    eb = eb * (pos[None, None, :] <= pos[None, :, None])  # zero k>q
    ebT = np.transpose(eb, (0, 2, 1))             # [h, key, query]
    # ebrep2[64c + kk, r, 4*hh + t, q] = ebT[r + 4*hh][kk, q]
    ebrep2 = np.empty((128, 4, 8, 64), dtype=np.float32)
    for r in range(4):
        for hh in range(2):
            for t in range(4):
                ebrep2[0:64, r, 4 * hh + t, :] = ebT[r + 4 * hh]
                ebrep2[64:128, r, 4 * hh + t, :] = ebT[r + 4 * hh]

    XDT = FP8 if USE_DR else BF16
    xt_fp8 = xt.astype(XDT)
    wqk_fp8 = np.ascontiguousarray(wqk.astype(XDT))
    wv_fp8 = np.ascontiguousarray(wv.astype(XDT))
    wp_bf = np.ascontiguousarray(wp.astype(BF16))
    ebrep2_bf = np.ascontiguousarray(ebrep2.astype(BF16))
    ident_bf = np.eye(128, dtype=np.float32).astype(BF16)

    in_maps = []
    for m in range(NCORES):
        sl = xt_fp8[:, :, m * TOK: (m + 1) * TOK]   # [2, 128, 9216]
        # device layout [128, 2, TOK]
        xm = np.ascontiguousarray(np.transpose(sl, (1, 0, 2)))
        im = {
            "xT": xm,
            "wqk": wqk_fp8,
            "wv": wv_fp8,
            "wp": wp_bf,
            "bq": bq64,
            "bp": bp,
            "ebrep2": ebrep2_bf,
        }
        if not USE_DMAT:
            im["ident"] = ident_bf
        in_maps.append(im)
    return in_maps


def _emit(nc, tc, d):
    """Emit the Tile kernel. d: dict of dram APs."""
    import concourse.bass as bass
    import concourse.mybir as mybir

    f32 = mybir.dt.float32
    bf16 = mybir.dt.bfloat16
    fp8 = mybir.dt.float8e4
    AFT = mybir.ActivationFunctionType
    DR = mybir.MatmulPerfMode.DoubleRow

    ctx = tc._emit_ctx  # ExitStack installed by caller

    consts = ctx.enter_context(tc.tile_pool(name="consts", bufs=1))
    persist = ctx.enter_context(tc.tile_pool(name="persist", bufs=1))
    sb_qk = ctx.enter_context(tc.tile_pool(name="sb_qk", bufs=3))
    sb_p = ctx.enter_context(tc.tile_pool(name="sb_p", bufs=3))
    sb_av = ctx.enter_context(tc.tile_pool(name="sb_av", bufs=3))
    sb_o = ctx.enter_context(tc.tile_pool(name="sb_o", bufs=2))
    # PSUM: qkv 1-bank tiles rotate through one bank (off the critical path);
    # S holds 4 banks (its four concurrent PE row-groups must drain into four
    # different banks); AV r-pair tiles rotate through 2 banks; projection
    # 1-bank tiles rotate through 1.  Total = 8.  Keeping AV and the
    # projection in separate pools decouples round R+1's AV matmuls from
    # round R's output evacuation.
    ps_qkv = ctx.enter_context(tc.tile_pool(name="ps_qkv", bufs=1, space="PSUM"))
    ps_s = ctx.enter_context(tc.tile_pool(name="ps_s", bufs=1, space="PSUM"))
    ps_av = ctx.enter_context(tc.tile_pool(name="ps_av", bufs=1, space="PSUM"))
    ps_o = ctx.enter_context(tc.tile_pool(name="ps_o", bufs=1, space="PSUM"))

    xdt = fp8 if USE_DR else bf16
    # ---- load constants ----
    wqk_sb = consts.tile([128, 2, 512], xdt)
    nc.sync.dma_start(wqk_sb, d["wqk"])
    wv_sb = consts.tile([128, 2, 256], xdt)
    nc.sync.dma_start(wv_sb, d["wv"])
    wp_sb = consts.tile([128, 2, 256], bf16)
    nc.sync.dma_start(wp_sb, d["wp"])
    bq_sb = consts.tile([128, 2], f32)
    nc.sync.dma_start(bq_sb, d["bq"])
    bp_sb = consts.tile([128, 2], f32)
    nc.sync.dma_start(bp_sb, d["bp"])
    ebrep_sb = consts.tile([128, 4, 8, 64], bf16)
    nc.sync.dma_start(ebrep_sb, d["ebrep2"])
    if not USE_DMAT:
        ident_sb = consts.tile([128, 128], bf16)
        nc.sync.dma_start(ident_sb, d["ident"])

    # ---- persistent xT (full residency, fp8) ----
    xT_sb = persist.tile([128, 2, TOK], xdt, name="xT_sb")
    for i in range(8):
        sl = slice(i * TOK // 8, (i + 1) * TOK // 8)
        nc.sync.dma_start(xT_sb[:, :, sl], d["xT"][:, :, sl])

    # ---- persistent V ([tok-in-128block, round, pair, head, 33]) ----
    v_all = persist.tile([128, NR, 4, HEADS, 33], bf16, name="v_all")
    nc.vector.memset(
        v_all.rearrange("p a b c e -> p (a b) c e")[:, :, :, 32:33], W8
    )

    def bcast(ap, n):
        return bass.AP(ap.tensor, ap.offset, [list(x) for x in ap.ap] + [[0, n]])

    def qkv_phase(R):
        tok0 = 512 * R
        rhs_x = xT_sb[:, :, tok0: tok0 + 512]
        # ---- Q, K: one DoubleRow matmul per 128-channel half ----
        q_sb = sb_qk.tile([128, 2, 512], bf16, name="q_sb", tag="qsb")
        k_sb = sb_qk.tile([128, 2, 512], bf16, name="k_sb", tag="ksb")
        for fc in range(2):
            q_ps = ps_qkv.tile([128, 512], f32, name=f"qps{fc}", tag="qkvps")
            if USE_DR:
                nc.tensor.matmul(
                    q_ps, wqk_sb[:, :, 128 * fc: 128 * (fc + 1)],
                    rhs_x, start=True, stop=True, perf_mode=DR,
                )
            else:
                for kc in range(2):
                    nc.tensor.matmul(
                        q_ps, wqk_sb[:, kc, 128 * fc: 128 * (fc + 1)],
                        rhs_x[:, kc, :], start=(kc == 0), stop=(kc == 1),
                    )
            nc.vector.tensor_scalar_add(q_sb[:, fc, :], q_ps, bq_sb[:, fc: fc + 1])
        for fc in range(2):
            k_ps = ps_qkv.tile([128, 512], f32, name=f"kps{fc}", tag="qkvps")
            if USE_DR:
                nc.tensor.matmul(
                    k_ps, wqk_sb[:, :, 256 + 128 * fc: 256 + 128 * (fc + 1)],
                    rhs_x, start=True, stop=True, perf_mode=DR,
                )
            else:
                for kc in range(2):
                    nc.tensor.matmul(
                        k_ps, wqk_sb[:, kc, 256 + 128 * fc: 256 + 128 * (fc + 1)],
                        rhs_x[:, kc, :], start=(kc == 0), stop=(kc == 1),
                    )
            nc.scalar.activation(k_sb[:, fc, :], k_ps, AFT.Copy)
        # ---- V: out [tok, ch], one DR matmul per 128 tokens ----
        for half in range(2):
            v_ps = ps_qkv.tile([128, 2, 256], f32, name=f"vps{half}", tag="qkvps")
            for b in range(2):
                if USE_DR:
                    nc.tensor.matmul(
                        v_ps[:, b, :],
                        xT_sb[:, :, tok0 + 256 * half + 128 * b: tok0 + 256 * half + 128 * (b + 1)],
                        wv_sb, start=True, stop=True, perf_mode=DR,
                    )
                else:
                    for kc in range(2):
                        nc.tensor.matmul(
                            v_ps[:, b, :],
                            xT_sb[:, kc, tok0 + 256 * half + 128 * b: tok0 + 256 * half + 128 * (b + 1)],
                            wv_sb[:, kc, :], start=(kc == 0), stop=(kc == 1),
                        )
            nc.scalar.activation(
                v_all[:, R, 2 * half: 2 * (half + 1), :, 0:32],
                v_ps.rearrange("p b (h e) -> p b h e", h=8),
                AFT.Copy,
            )
        return q_sb, k_sb

    st = {}   # per-round tile state

    def s_phase(R):
        # ---- S^T blocks: out [64 key, 64 q] per (pos, head), 8-way packed.
        # s_ps[64c + key, r = h%4 (bank!), 4*(h//4) + s//2, q] -- the four
        # concurrently-draining row-groups land in four different banks.
        q_sb, k_sb = st[R]["qk"]
        s_ps = ps_s.tile([128, 4, 8, 64], f32, name="s_ps", tag="sps")
        for s in range(8):
            c, t = s % 2, s // 2
            for h in range(HEADS):
                r, hh = h % 4, h // 4
                nc.tensor.matmul(
                    s_ps[64 * c: 64 * (c + 1), r, 4 * hh + t, :],
                    k_sb[32 * r: 32 * r + 32, hh, 64 * s: 64 * (s + 1)],
                    q_sb[32 * r: 32 * r + 32, hh, 64 * s: 64 * (s + 1)],
                    start=True, stop=True,
                    tile_position=(32 * r, 64 * c),
                )
        # ---- exp over r-pairs (scale folds away the fp8 weight prescale) --
        et = sb_p.tile([128, 4, 8, 64], bf16, name="et", tag="et")
        for j in range(2):
            nc.scalar.activation(
                et[:, 2 * j: 2 * (j + 1), :, :].rearrange("p a b e -> p (a b e)"),
                s_ps[:, 2 * j: 2 * (j + 1), :, :].rearrange("p a b e -> p (a b e)"),
                AFT.Exp, scale=float(1.0 / (W8 * W8)),
            )
        st[R]["et"] = et

    def mul_phase(R):
        # ---- * exp(rel-pos bias) * causal mask, on GpSimd (a full
        # iteration of slack before AV(R) consumes pt) ----
        et = st[R].pop("et")
        pt = sb_p.tile([128, 4, 8, 64], bf16, name="pt", tag="pt")
        eng = nc.gpsimd if USE_GPS else nc.vector
        for j in range(2):
            eng.tensor_mul(
                pt[:, 2 * j: 2 * (j + 1), :, :].rearrange("p a b e -> p (a b e)"),
                et[:, 2 * j: 2 * (j + 1), :, :].rearrange("p a b e -> p (a b e)"),
                ebrep_sb[:, 2 * j: 2 * (j + 1), :, :].rearrange("p a b e -> p (a b e)"),
            )
        st[R]["pt"] = pt

    def av_phase(R):
        # ---- AV per r-pair: out [64 q, 33] per (pos, head), 2-way diag ----
        # av_j[64c + q, rbit, 2*t + hh, 0:33], bank = rbit.  The slot order
        # (2t+hh) makes (t, hh) a single uniform-stride dim in the normalize
        # APs (TENSOR3D allows at most 3 free dims).
        pt = st[R].pop("pt")
        rsb = sb_av.tile([128, 2, 16], f32, name="rsb", tag="rsb")
        avn = sb_av.tile([128, 4, 2, 4, 32], bf16, name="avn", tag="avn")
        avs = []
        for j in range(2):
            av_j = ps_av.tile([128, 2, 8, 64], f32, name=f"av{j}", tag="avps")
            avs.append(av_j)
            for s in range(8):
                c, t = s % 2, s // 2
                for hh in range(2):
                    for rbit in range(2):
                        h = 2 * j + rbit + 4 * hh
                        nc.tensor.matmul(
                            av_j[64 * c: 64 * (c + 1), rbit, 2 * t + hh, 0:33],
                            pt[64 * c: 64 * (c + 1), 2 * j + rbit, 4 * hh + t, :],
                            v_all[64 * c: 64 * (c + 1), R, t, h, :],
                            start=True, stop=True,
                            tile_position=(64 * c, 64 * c),
                        )
        for j in range(2):
            av_j = avs[j]
            # ---- denominators -> reciprocal ----
            nc.vector.reciprocal(
                rsb[:, j, :].rearrange("p (a b) -> p a b", a=2), av_j[:, :, :, 32]
            )
            # ---- normalize into avn[tok, (t, w=hh, h3, hd)], h3 = 2j+rbit --
            # avn offset(rbit, g=2t+hh, hd) = 64j + 32*rbit + 128*g + hd
            nc.vector.tensor_mul(
                bass.AP(avn.tensor, avn.offset + 64 * j,
                        [list(avn.ap[0]), [32, 2], [128, 8], [1, 32]]),
                av_j[:, :, :, 0:32],
                bass.AP(rsb.tensor, rsb.offset + 16 * j,
                        [list(rsb.ap[0]), [8, 2], [1, 8], [0, 32]]),
            )
        st[R]["avn"] = avn

    def tr_phase(R):
        # ---- transpose via DMA xbar: avt[ch, (t, w), tok128] ----
        avn = st[R].pop("avn")
        avt = sb_av.tile([128, 8, 128], bf16, name="avt", tag="avt")
        nc.sync.dma_start_transpose(avt, avn.rearrange("p a b c e -> p (a b c e)"))
        st[R]["avt"] = avt

    def out_phase(R):
        # ---- projection (bf16, 2-step accumulate) + bias + store ----
        tok0 = 512 * R
        avt = st.pop(R)["avt"]
        pps = ps_s.tile([128, 2, 512], f32, name="pps", tag="sps")
        osb = sb_o.tile([128, 2, 512], bf16, name="osb", tag="osb")
        for ec in range(2):
            for w in range(2):
                nc.tensor.matmul(
                    pps[:, ec, :], wp_sb[:, w, 128 * ec: 128 * (ec + 1)],
                    bass.AP(avt.tensor, avt.offset + 128 * w,
                            [list(avt.ap[0]), [256, 4], [1, 128]]),
                    start=(w == 0), stop=(w == 1),
                )
        for ec in range(2):
            nc.vector.tensor_scalar_add(osb[:, ec, :], pps[:, ec, :], bp_sb[:, ec: ec + 1])
        nc.gpsimd.dma_start(
            d["outT"][:, tok0: tok0 + 512].rearrange("(e p) t -> p e t", e=2),
            osb,
        )

    # 3-deep software pipeline: every instruction's dependencies are at
    # least one iteration old when it reaches its (in-order) engine queue.
    st[0] = {"qk": qkv_phase(0)}
    for i in range(NR + 2):
        if i + 1 < NR:
            st[i + 1] = {"qk": qkv_phase(i + 1)}
        if i < NR:
            s_phase(i)
        if 0 <= i - 1 < NR:
            av_phase(i - 1)
        if i < NR:
            mul_phase(i)
        if 0 <= i - 1 < NR:
            tr_phase(i - 1)
        if 0 <= i - 2 < NR:
            out_phase(i - 2)


def build():
    """Build + compile the Bass program (cached)."""
    if "nc" in _BUILD_CACHE:
        return _BUILD_CACHE["nc"]
    import concourse.mybir as mybir
    import concourse.tile as tile
    from concourse import bacc

    f32 = mybir.dt.float32
    bf16 = mybir.dt.bfloat16
    fp8 = mybir.dt.float8e4

    nc = bacc.Bacc("TRN2", target_bir_lowering=False, debug=False,
                   enable_asserts=False, num_devices=NCORES)
    xdt = fp8 if USE_DR else bf16
    d = {
        "xT": nc.dram_tensor("xT", [128, 2, TOK], xdt, kind="ExternalInput").ap(),
        "wqk": nc.dram_tensor("wqk", [128, 2, 512], xdt, kind="ExternalInput").ap(),
        "wv": nc.dram_tensor("wv", [128, 2, 256], xdt, kind="ExternalInput").ap(),
        "wp": nc.dram_tensor("wp", [128, 2, 256], bf16, kind="ExternalInput").ap(),
        "bq": nc.dram_tensor("bq", [128, 2], f32, kind="ExternalInput").ap(),
        "bp": nc.dram_tensor("bp", [128, 2], f32, kind="ExternalInput").ap(),
        "ebrep2": nc.dram_tensor("ebrep2", [128, 4, 8, 64], bf16, kind="ExternalInput").ap(),
        "outT": nc.dram_tensor("outT", [256, TOK], bf16, kind="ExternalOutput").ap(),
    }
    if not USE_DMAT:
        d["ident"] = nc.dram_tensor("ident", [128, 128], bf16, kind="ExternalInput").ap()
    with tile.TileContext(nc) as tc:
        with ExitStack() as es:
            tc._emit_ctx = es
            _emit(nc, tc, d)
    nc.compile()
    _BUILD_CACHE["nc"] = nc
    return nc


def _install_ntff_hook():
    """Provide antenv.axon_hooks with a ctypes NTFF profiling hook if the
    image's antenv package lacks it (mirrors the agent-boot registration)."""
    import contextlib
    import ctypes
    import types

    try:
        from antenv.axon_hooks import get_axon_ntff_profile_hook  # noqa: F401
        return True
    except ImportError:
        pass
    so_path = "/opt/axon/libaxon_pjrt.so"
    if not os.path.exists(so_path):
        return False
    lib = ctypes.CDLL(so_path)
    if not hasattr(lib, "axon_start_nrt_profile"):
        return False
    lib.axon_start_nrt_profile.argtypes = [ctypes.POINTER(ctypes.c_int64), ctypes.c_size_t]
    lib.axon_start_nrt_profile.restype = ctypes.c_int64
    lib.axon_stop_nrt_profile.argtypes = [ctypes.c_char_p]
    lib.axon_stop_nrt_profile.restype = ctypes.c_int64

    @contextlib.contextmanager
    def _hook(output_dir, device_ids):
        import jax
        jax.devices()
        if device_ids:
            ids = (ctypes.c_int64 * len(device_ids))(*device_ids)
            rc = lib.axon_start_nrt_profile(ids, len(device_ids))
        else:
            rc = lib.axon_start_nrt_profile(None, 0)
        if rc != 0:
            raise RuntimeError(f"axon_start_nrt_profile rc={rc}")
        try:
            yield
        finally:
            n = lib.axon_stop_nrt_profile(str(output_dir).encode())
            print(f"profile: {n} file(s) written to {output_dir}", file=sys.stderr)

    import antenv
    mod = types.ModuleType("antenv.axon_hooks")
    _state = {"hook": _hook}
    mod.get_axon_ntff_profile_hook = lambda: _state["hook"]
    mod.set_axon_ntff_profile_hook = lambda h: _state.update(hook=h)
    sys.modules["antenv.axon_hooks"] = mod
    antenv.axon_hooks = mod
    return True


def kernel(x, rpb_table, qkv_w, qkv_b, proj_w, proj_b):
    in_maps = _host_prep(x, rpb_table, qkv_w, qkv_b, proj_w, proj_b)
    nc = build()
    from concourse import bass_utils

    trace = bool(int(os.environ.get("BASS_KERNEL_TRACE", "0")))
    if trace:
        trace = _install_ntff_hook()
    try:
        res = bass_utils.run_bass_kernel_spmd(
            nc, in_maps, core_ids=list(range(NCORES)), trace=trace
        )
    except Exception:
        if not trace:
            raise
        import traceback
        traceback.print_exc()
        print("trace run failed; retrying without trace", file=sys.stderr)
        res = bass_utils.run_bass_kernel_spmd(
            nc, in_maps, core_ids=list(range(NCORES)), trace=False
        )
    if trace and res.exec_time_ns is not None:
        print(f"HW exec time: {res.exec_time_ns} ns")
        _BUILD_CACHE["exec_time_ns"] = res.exec_time_ns
        _BUILD_CACHE["profile_res"] = res

    x = np.asarray(x, dtype=np.float32)
    out = np.empty_like(x)
    out[:, :, :HALF] = x[:, :, :HALF]
    # outT per core: [256, 9216] bf16 -> positions
    attn = np.empty((HALF, NPOS, T), dtype=np.float32)
    for m in range(NCORES):
        o = np.asarray(res.results[m]["outT"], dtype=np.float32)
        attn[:, m * POS_PER_CORE: (m + 1) * POS_PER_CORE, :] = o.reshape(
            HALF, POS_PER_CORE, T
        )
    # (c, B, H, W, T) -> (B, T, c, H, W)
    attn = attn.reshape(HALF, B, HS, WS, T)
    out[:, :, HALF:] = np.transpose(attn, (1, 4, 0, 2, 3))
    return out
